# revision 2
# baseline (speedup 1.0000x reference)
"""Trainium2 Bass kernel for MemoryEfficientCrossAttention (v2).

Problem (hardcoded): B=2, Q=2048, K=4096, HIDDEN=1024, HEADS=16, HEAD_DIM=64.
  out = softmax((x_q W_q)(x_k W_k)^T / sqrt(64)) (x_v W_v) W_o

Sharding over 8 NeuronCores: core = b*4 + hg
  b in {0,1}: batch;  hg in {0..3}: head-group of 4 heads (256 cols of W_qkv)
Each core projects q/k/v for its batch+heads (bf16), runs attention for its
4 heads over all 2048 q, then the 4-core batch group
AllGathers normalized context per q-slice and every core applies its own
256-column slice of W_o to all 2048 q rows (host reassembles columns).
No duplicated FLOPs anywhere.

Engine plan per core: ACT does 33.5M exps (the bottleneck); PE does bf16
matmuls (scores contraction=64 auto-pair onto 64x128 row tiles T0/T8, two
heads concurrent); DVE converts/evacuates/spills; Pool runs the collectives.
Attention runs in 3 kb-windows [0-8/8-20/20-32) so exp starts after 1/4 of
k/v is loaded; per-window partial ctx (ones-column denominator in row 64)
spills from PSUM into an SBUF accumulator.  In the last window each q-slice
is normalized (reciprocal-approx + DMA-partition-broadcast of 1/den) and
AllGathered as soon as its units finish, overlapping collectives and the
column-sharded W_o with remaining attention.

PSUM budget: 2x score tiles [128,1024] (4 banks) + 2x ctx [65,512] (2) +
2x projection/wo [128,512] (2) = 8 banks exactly.
"""

import os
import sys
import time

import numpy as np

sys.path.insert(0, "/opt/trn_rl_repo")

import concourse.bass as bass  # noqa: E402
import concourse.mybir as mybir  # noqa: E402
import concourse.tile as tile  # noqa: E402
from concourse import bacc  # noqa: E402

F32 = mybir.dt.float32
F32R = mybir.dt.float32r
BF16 = mybir.dt.bfloat16

HID = 1024
HEADS = 16
HD = 64
B = 2
Q = 2048
KL = 4096
NCORE = 8
HPC = 4           # heads per core
GC = HPC * HD     # 256 proj cols per core
QS = 512          # q rows per output slice / attention q block
NQB = Q // QS     # 4
NKB = KL // 128   # 32 k-blocks
NCH = HID // 128   # 8 hidden chunks
SCALE = HD ** -0.5

_CACHED_NC = None


def _build():
    nc = bacc.Bacc("TRN2", target_bir_lowering=False, debug=False,
                   num_devices=NCORE)

    q_rows = nc.dram_tensor("q_rows", [Q, HID], F32, kind="ExternalInput")
    key_b = nc.dram_tensor("key_b", [KL, HID], F32, kind="ExternalInput")
    value_b = nc.dram_tensor("value_b", [KL, HID], F32, kind="ExternalInput")
    wq_s = nc.dram_tensor("wq_s", [HID, GC], F32, kind="ExternalInput")
    wk_s = nc.dram_tensor("wk_s", [HID, GC], F32, kind="ExternalInput")
    wv_s = nc.dram_tensor("wv_s", [HID, GC], F32, kind="ExternalInput")
    w_o = nc.dram_tensor("w_o", [HID, GC], F32, kind="ExternalInput")
    outT = nc.dram_tensor("outT", [GC, Q], F32, kind="ExternalOutput")
    debug = bool(int(os.environ.get("KDEBUG", "0")))
    if debug:
        dbg_qT = nc.dram_tensor("dbg_qT", [GC, Q], F32, kind="ExternalOutput")
        dbg_kT = nc.dram_tensor("dbg_kT", [GC, KL], F32, kind="ExternalOutput")
        dbg_v = nc.dram_tensor("dbg_v", [KL, GC], F32, kind="ExternalOutput")
        dbg_ctx = nc.dram_tensor("dbg_ctx", [GC, Q], F32, kind="ExternalOutput")
        dbg_acc = nc.dram_tensor("dbg_acc", [128, HPC * NQB * QS], F32,
                                 kind="ExternalOutput")

    with tile.TileContext(nc, pool_alloc_mode="queue") as tc:
        with tc.tile_pool(name="dram", bufs=1, space="DRAM") as dram:
            from contextlib import ExitStack
            st = ExitStack()
            pp = st.enter_context(tc.tile_pool(name="persist", bufs=1))
            xrow = st.enter_context(tc.tile_pool(name="xrow", bufs=3))
            xbf = st.enter_context(tc.tile_pool(name="xbf", bufs=4))
            xts = st.enter_context(tc.tile_pool(name="xT", bufs=4))
            wst = st.enter_context(tc.tile_pool(name="wstage", bufs=2))
            ctxgp = st.enter_context(tc.tile_pool(name="ctxg", bufs=10))
            otp = st.enter_context(tc.tile_pool(name="ot", bufs=2))
            apool = st.enter_context(tc.tile_pool(name="aring", bufs=6))
            misc = st.enter_context(tc.tile_pool(name="misc", bufs=2))
            pproj = st.enter_context(
                tc.tile_pool(name="pproj", bufs=2, space="PSUM"))
            pscore = st.enter_context(
                tc.tile_pool(name="pscore", bufs=2, space="PSUM"))
            pctx = st.enter_context(
                tc.tile_pool(name="pctx", bufs=1, space="PSUM"))

            ctx_stage = dram.tile([NQB, 2, 128, QS], BF16)
            ctx_gath = dram.tile([NQB, NCH, 128, QS], BF16)
            rdram = dram.tile([HPC * NQB, 1, QS], F32)

            qT = pp.tile([128, 2, Q], BF16)
            kT = pp.tile([128, 2, KL], BF16)
            v_aug = pp.tile([128, NKB, HPC, HD + 1], BF16)
            wq_b = pp.tile([128, NCH, GC], BF16)
            wk_b = pp.tile([128, NCH, GC], BF16)
            wv_b = pp.tile([128, NCH, GC], BF16)
            wo_b = pp.tile([128, NCH, GC], BF16)
            ctxacc = pp.tile([128, HPC * NQB, QS], F32)
            ctxn = pp.tile([128, 2, NQB, QS], BF16)
            ones64 = pp.tile([1, HD], F32)

            nc.vector.memset(v_aug[:], 1.0)
            nc.vector.memset(ones64[:], 1.0)

            # ---------------- small weights ----------------
            for hc in range(NCH):
                hsl = slice(hc * 128, (hc + 1) * 128)
                for wdram, wsb in ((wq_s, wq_b), (wk_s, wk_b), (wv_s, wv_b),
                                   (w_o, wo_b)):
                    ws = wst.tile([128, GC], F32, tag="w")
                    nc.gpsimd.dma_start(ws[:], wdram[hsl, :])
                    nc.vector.tensor_copy(wsb[:, hc, :], ws[:])

            # ---------------- activation transpose helper ----------------
            def stage_chunk(src, row0, tag):
                """Transpose 512 rows of src into [128, NCH, 512] bf16."""
                xT = xts.tile([128, NCH, 512], BF16, tag="xT")
                for t in range(4):
                    r0 = row0 + t * 128
                    xf = xrow.tile([128, HID], F32, tag="xf")
                    nc.sync.dma_start(xf[:], src[r0:r0 + 128, :])
                    xb = xbf.tile([128, HID], BF16, tag="xb")
                    nc.vector.tensor_copy(xb[:], xf[:])
                    sl = slice(t * 128, (t + 1) * 128)
                    nc.sync.dma_start_transpose(xT[:, :, sl], xb[:])
                return xT

            # ---------------- q projection (all chunks) ----------------
            for qc in range(Q // 512):
                xT = stage_chunk(q_rows, qc * 512, "q")
                for hp in range(2):
                    ps = pproj.tile([128, 512], F32, tag="pp")
                    for hc in range(NCH):
                        nc.tensor.matmul(
                            ps[:],
                            wq_b[:, hc, hp * 128:(hp + 1) * 128],
                            xT[:, hc, :],
                            start=(hc == 0), stop=(hc == NCH - 1))
                    nc.vector.tensor_copy(
                        qT[:, hp, qc * 512:(qc + 1) * 512], ps[:])

            # ---------------- windows: k/v projection + attention --------
            WINDOWS = [(0, 8), (8, 20), (20, 32)]
            for w, (kb0, kb1) in enumerate(WINDOWS):
                for kc in range(kb0 // 4, kb1 // 4):
                    xT = stage_chunk(key_b, kc * 512, "k")
                    for hp in range(2):
                        ps = pproj.tile([128, 512], F32, tag="pp")
                        for hc in range(NCH):
                            nc.tensor.matmul(
                                ps[:],
                                wk_b[:, hc, hp * 128:(hp + 1) * 128],
                                xT[:, hc, :],
                                start=(hc == 0), stop=(hc == NCH - 1))
                        nc.vector.tensor_copy(
                            kT[:, hp, kc * 512:(kc + 1) * 512], ps[:])
                    xT = stage_chunk(value_b, kc * 512, "v")
                    for t in range(4):
                        ps = pproj.tile([128, 512], F32, tag="pp")
                        nc.tensor.matmul(
                            ps[:, 0:GC],
                            xT[:, 0, t * 128:(t + 1) * 128],
                            wv_b[:, 0, :],
                            start=True, stop=False)
                        for hc in range(1, NCH):
                            nc.tensor.matmul(
                                ps[:, 0:GC],
                                xT[:, hc, t * 128:(t + 1) * 128],
                                wv_b[:, hc, :],
                                start=False, stop=(hc == NCH - 1))
                        nc.vector.tensor_copy(
                            v_aug[:, kc * 4 + t, :, 0:HD],
                            ps[:, 0:GC].rearrange("p (h d) -> p h d", h=HPC))

                for qb in range(NQB):
                    qsl = slice(qb * QS, (qb + 1) * QS)
                    for hp in range(2):
                        ctxs = [pctx.tile([HD + 1, QS], F32, tag=f"ctx{i}",
                                          name=f"ctx{i}_{w}_{qb}_{hp}")
                                for i in range(2)]
                        for j, kb in enumerate(range(kb0, kb1)):
                            ksl = slice(kb * 128, (kb + 1) * 128)
                            sts = pscore.tile([128, 2 * QS], F32, tag="sts")
                            for i in range(2):
                                psl = slice(i * 64, (i + 1) * 64)
                                nc.tensor.matmul(
                                    sts[:, i * QS:(i + 1) * QS],
                                    kT[psl, hp, ksl],
                                    qT[psl, hp, qsl],
                                    start=True, stop=True)
                            actx = apool.tile([128, 2, QS], BF16, tag="actx",
                                              name=f"actx_{w}_{qb}_{hp}_{j}")
                            nc.scalar.activation(
                                actx[:], sts[:],
                                mybir.ActivationFunctionType.Exp,
                                scale=SCALE)
                            for i in range(2):
                                nc.tensor.matmul(
                                    ctxs[i][:],
                                    v_aug[:, kb, 2 * hp + i, :],
                                    actx[:, i, :],
                                    start=(j == 0),
                                    stop=(kb == kb1 - 1))
                        for i in range(2):
                            uid = qb * HPC + hp * 2 + i
                            if w == 0:
                                nc.vector.tensor_copy(
                                    ctxacc[0:HD + 1, uid, :],
                                    ctxs[i][0:HD + 1, :])
                            else:
                                nc.vector.tensor_add(
                                    ctxacc[0:HD + 1, uid, :],
                                    ctxs[i][0:HD + 1, :],
                                    ctxacc[0:HD + 1, uid, :])

                    # normalize + gather this q-slice as soon as its last
                    # window is done, so collectives overlap the remaining
                    # attention instead of serializing at the end.
                    if w == len(WINDOWS) - 1:
                        for hp in range(2):
                            for i in range(2):
                                uid = qb * HPC + hp * 2 + i
                                den0 = misc.tile([1, QS], F32, tag="den0",
                                                 name=f"den0_{uid}")
                                nc.vector.tensor_copy(
                                    den0[:], ctxacc[HD:HD + 1, uid, :])
                                rinv = misc.tile([1, QS], F32, tag="rinv",
                                                 name=f"rinv_{uid}")
                                nc.vector.reciprocal_approx_fast(
                                    rinv[:], den0[:])
                                nc.sync.dma_start(rdram[uid], rinv[:])
                                rb = misc.tile([HD, QS], F32, tag="rb",
                                               name=f"rb_{uid}")
                                nc.sync.dma_start(
                                    rb[:], rdram[uid].partition_broadcast(HD))
                                nc.vector.tensor_mul(
                                    ctxn[i * HD:(i + 1) * HD, hp, qb, :],
                                    ctxacc[0:HD, uid, :], rb[:])
                            nc.sync.dma_start(ctx_stage[qb, hp],
                                              ctxn[:, hp, qb, :])
                        nc.gpsimd.collective_compute(
                            "AllGather", mybir.AluOpType.bypass,
                            ins=[ctx_stage[qb]],
                            outs=[ctx_gath[qb]],
                            replica_groups=[[0, 1, 2, 3], [4, 5, 6, 7]])

            if debug:
                for hp in range(2):
                    nc.gpsimd.dma_start(
                        dbg_qT[hp * 128:(hp + 1) * 128, :], qT[:, hp, :])
                    nc.gpsimd.dma_start(
                        dbg_kT[hp * 128:(hp + 1) * 128, :], kT[:, hp, :])
                for kb in range(NKB):
                    nc.gpsimd.dma_start(
                        dbg_v[kb * 128:(kb + 1) * 128, :].rearrange(
                            "p (h d) -> p h d", h=HPC),
                        v_aug[:, kb, :, 0:HD])
                for qb in range(NQB):
                    for hp in range(2):
                        nc.gpsimd.dma_start(
                            dbg_ctx[hp * 128:(hp + 1) * 128,
                                    qb * QS:(qb + 1) * QS],
                            ctxn[:, hp, qb, :])
                nc.gpsimd.dma_start(
                    dbg_acc[:].rearrange("p (u q) -> p u q", u=HPC * NQB),
                    ctxacc[:])

            # ---------------- output projection ----------------
            for qb in range(NQB):
                ctxg = [ctxgp.tile([128, QS], BF16, tag="ctxg",
                                   name=f"ctxg_{qb}_{hc}")
                        for hc in range(NCH)]
                for hc in range(NCH):
                    nc.sync.dma_start(ctxg[hc][:], ctx_gath[qb, hc])
                for oc in range(GC // 128):
                    po = pproj.tile([128, 512], F32, tag="pp",
                                    name=f"po_{qb}_{oc}")
                    for hc in range(NCH):
                        nc.tensor.matmul(
                            po[:],
                            wo_b[:, hc, oc * 128:(oc + 1) * 128],
                            ctxg[hc][:],
                            start=(hc == 0), stop=(hc == NCH - 1))
                    ot = otp.tile([128, 512], F32, tag="ot",
                                  name=f"ot_{qb}_{oc}")
                    nc.vector.tensor_copy(ot[:], po[:])
                    nc.sync.dma_start(
                        outT[oc * 128:(oc + 1) * 128,
                             qb * QS:(qb + 1) * QS], ot[:])

            st.close()

    nc.compile()
    return nc


def _get_nc():
    global _CACHED_NC
    if _CACHED_NC is None:
        _CACHED_NC = _build()
    return _CACHED_NC


def make_in_maps(query, key, value, w_q, w_k, w_v, w_o):
    ins = []
    for core in range(NCORE):
        b, hg = core // 4, core % 4
        csl = slice(hg * GC, (hg + 1) * GC)
        ins.append({
            "q_rows": np.ascontiguousarray(query[b]),
            "key_b": np.ascontiguousarray(key[b]),
            "value_b": np.ascontiguousarray(value[b]),
            "wq_s": np.ascontiguousarray(w_q[:, csl]),
            "wk_s": np.ascontiguousarray(w_k[:, csl]),
            "wv_s": np.ascontiguousarray(w_v[:, csl]),
            "w_o": np.ascontiguousarray(w_o[:, csl]),
        })
    return ins


def assemble(results):
    out = np.empty((B, Q, HID), np.float32)
    for core in range(NCORE):
        b, hg = core // 4, core % 4
        out[b, :, hg * GC:(hg + 1) * GC] = results[core]["outT"].T
    return out


_EXEC = None


def _get_exec():
    """Build the 8-core shard_map executable once; reuse across calls."""
    global _EXEC
    if _EXEC is not None:
        return _EXEC
    import jax
    from jax.sharding import Mesh, PartitionSpec
    from jax.experimental.shard_map import shard_map
    from concourse.bass2jax import (_bass_exec_p, install_neuronx_cc_hook,
                                    partition_id_tensor)

    install_neuronx_cc_hook()
    nc = _get_nc()
    in_names, out_names, out_avals, zero_outs = [], [], [], []
    for alloc in nc.m.functions[0].allocations:
        if not isinstance(alloc, mybir.MemoryLocationSet):
            continue
        name = alloc.memorylocations[0].name
        if alloc.kind == "ExternalInput":
            if name != "partition_id":
                in_names.append(name)
        elif alloc.kind == "ExternalOutput":
            out_names.append(name)
            shape = tuple(alloc.tensor_shape)
            dtype = mybir.dt.np(alloc.dtype)
            out_avals.append(jax.core.ShapedArray(shape, dtype))
            zero_outs.append(np.zeros(shape, dtype))
    partition_name = (nc.partition_id_tensor.name
                      if nc.partition_id_tensor else None)
    all_in = list(in_names) + list(out_names)
    if partition_name:
        all_in.append(partition_name)

    def _body(*args):
        operands = list(args)
        if partition_name is not None:
            operands.append(partition_id_tensor())
        return tuple(_bass_exec_p.bind(
            *operands, out_avals=tuple(out_avals), in_names=tuple(all_in),
            out_names=tuple(out_names), lowering_input_output_aliases=(),
            sim_require_finite=True, sim_require_nnan=True, nc=nc))

    devices = jax.devices()[:NCORE]
    mesh = Mesh(np.asarray(devices), ("core",))
    n_all = len(in_names) + len(out_names)
    fn = jax.jit(shard_map(_body, mesh=mesh,
                           in_specs=(PartitionSpec("core"),) * n_all,
                           out_specs=(PartitionSpec("core"),) * len(out_names),
                           check_rep=False), keep_unused=True)
    concat_zeros = [np.zeros((NCORE * z.shape[0], *z.shape[1:]), z.dtype)
                    for z in zero_outs]
    _EXEC = (fn, in_names, out_names, out_avals, concat_zeros)
    return _EXEC


def kernel(query, key, value, w_q, w_k, w_v, w_o):
    query = np.asarray(query, dtype=np.float32)
    key = np.asarray(key, dtype=np.float32)
    value = np.asarray(value, dtype=np.float32)
    ins = make_in_maps(query, key, value, np.asarray(w_q, np.float32),
                       np.asarray(w_k, np.float32), np.asarray(w_v, np.float32),
                       np.asarray(w_o, np.float32))
    fn, in_names, out_names, out_avals, concat_zeros = _get_exec()
    concat_in = [np.concatenate([np.asarray(ins[c][nm]) for c in range(NCORE)])
                 for nm in in_names]
    out_arrs = fn(*concat_in, *concat_zeros)
    results = [
        {nm: np.asarray(out_arrs[i]).reshape(NCORE, *out_avals[i].shape)[c]
         for i, nm in enumerate(out_names)}
        for c in range(NCORE)]
    return assemble(results)


if __name__ == "__main__":
    np.random.seed(0)
    q = np.random.randn(B, Q, HID).astype(np.float32)
    k = np.random.randn(B, KL, HID).astype(np.float32)
    v = np.random.randn(B, KL, HID).astype(np.float32)
    s = 1.0 / np.sqrt(HID)
    wq = (np.random.randn(HID, HID) * s).astype(np.float32)
    wk = (np.random.randn(HID, HID) * s).astype(np.float32)
    wv = (np.random.randn(HID, HID) * s).astype(np.float32)
    wo = (np.random.randn(HID, HID) * s).astype(np.float32)
    t0 = time.time()
    out = kernel(q, k, v, wq, wk, wv, wo)
    print("kernel done", time.time() - t0, out.shape)
